# revision 12
# baseline (speedup 1.0000x reference)
"""ARB loss kernel for Trainium2, SPMD across 8 NeuronCores.

Reference computation (n=8192 rows, C=32000 classes):
    counts = bincount(y, C)                       # label histogram
    w[i]   = counts[y[i]]
    rowsum[i] = sum_c output[i, c]
    denom[i]  = (n / w[i]) * rowsum[i]
    loss = -mean_i log(output[i, y[i]] / denom[i])
         = log(n) - (1/n) * sum_i log(output[i,y[i]] * w[i] / rowsum[i])

Sharding: data-parallel over rows, 1024 rows per core. Each core:
  - streams its 1024x32000 f32 shard (131 MB) through SBUF in
    [128 x 8000] tiles; row sums are computed on the fly, split between
    the Vector engine (reduce_sum over the first D_DVE cols) and the
    Scalar engine (activation Copy + accum_out over the rest) so
    neither engine paces the stream — the kernel is HBM-DMA bound.
  - computes w for its rows from the full label vector (replicated to
    every core, so no bincount all-reduce is needed): per 128-row block,
    tensor_scalar(is_equal) against the 8192-long label list with a
    fused add-reduction, split into halves interleaved with the stream.
  - gathers output[i, y[i]] with elementwise indirect DMA.
  - evaluates log(true*w) and log(rowsum) on the Scalar engine with a
    fused free-dim accumulation -> two partial sums per partition.
Host unshard: loss = log(n) - (sum(acc0) - sum(acc1))/n.
"""

import math
import sys
from contextlib import ExitStack

import numpy as np

if "/opt/trn_rl_repo" not in sys.path:
    sys.path.insert(0, "/opt/trn_rl_repo")

# bass_utils imports antenv.axon_hooks when BASS_TRACE is set; make sure a
# stub exists so a missing module never crashes the run (trace then simply
# degrades to no-profile).
try:
    import antenv.axon_hooks  # noqa: F401
except ImportError:
    import types

    try:
        import antenv

        _stub = types.ModuleType("antenv.axon_hooks")
        _stub._HOOK = None
        _stub.set_axon_ntff_profile_hook = lambda h: setattr(_stub, "_HOOK", h)
        _stub.get_axon_ntff_profile_hook = lambda: _stub._HOOK
        sys.modules["antenv.axon_hooks"] = _stub
        antenv.axon_hooks = _stub
    except ImportError:
        pass

N = 8192           # total rows
C = 32000          # classes
NCORES = 8
RPC = N // NCORES  # rows per core = 1024
P = 128            # partitions
RB = RPC // P      # row blocks per core = 8
COLCH = 8000       # columns per streamed tile
NCH = C // COLCH   # column chunks per row block = 4
NT = RB * NCH      # streamed tiles per core = 32
NBUF = 4           # stream buffers
D_DVE = 3072       # columns of each tile reduced on VectorE
# remaining COLCH - D_DVE columns reduced on ScalarE
NHALF = 2 * RB     # count half-ops (one per 2 tiles)
HALF = N // 2      # labels per count half-op

_CACHE = {}


def _build_nc():
    import concourse.bass as bass
    import concourse.mybir as mybir

    f32 = mybir.dt.float32
    i32 = mybir.dt.int32
    bf16 = mybir.dt.bfloat16
    D_ACT = COLCH - D_DVE

    nc = bass.Bass()
    x_ext = nc.dram_tensor("x", [RPC, C], f32, kind="ExternalInput")
    yf_ext = nc.dram_tensor("yf", [1, N], f32, kind="ExternalInput")
    ylf_ext = nc.dram_tensor("ylf", [P, RB], f32, kind="ExternalInput")
    off_ext = nc.dram_tensor("off", [P, RB], i32, kind="ExternalInput")
    out_ext = nc.dram_tensor("out", [P, 2], f32, kind="ExternalOutput")

    with ExitStack() as es:
        ec = es.enter_context
        data = [
            ec(nc.sbuf_tensor(f"data{j}", [P, COLCH], f32))
            for j in range(NBUF)
        ]
        yfb = ec(nc.sbuf_tensor([P, N], f32))
        yf_sb = ec(nc.sbuf_tensor([1, N], f32))
        ones_sb = ec(nc.sbuf_tensor([1, P], f32))
        eqscr = ec(nc.sbuf_tensor([P, HALF], bf16))
        act_scr = ec(nc.sbuf_tensor([P, D_ACT], bf16))
        rs_part = ec(nc.sbuf_tensor([P, NT + 1], f32))   # DVE partials per load
        act_part = ec(nc.sbuf_tensor([P, NT + 1], f32))  # ACT partials per load
        sum4 = ec(nc.sbuf_tensor([P, NCH + 1], f32))
        rs = ec(nc.sbuf_tensor([P, RB], f32))
        w_half = ec(nc.sbuf_tensor([P, NHALF], f32))
        w_sb = ec(nc.sbuf_tensor([P, RB], f32))
        tv = ec(nc.sbuf_tensor([P, RB], f32))
        ylf_sb = ec(nc.sbuf_tensor([P, RB], f32))
        off_sb = ec(nc.sbuf_tensor([P, RB], i32))
        tprod = ec(nc.sbuf_tensor([P, RB], f32))
        logt = ec(nc.sbuf_tensor([P, RB], f32))
        acc = ec(nc.sbuf_tensor([P, 2], f32))

        psum = [ec(nc.psum_tensor(f"bps{j}", [P, 512], f32)) for j in range(2)]
        dmaL = [ec(nc.semaphore(f"dmaL{j}")) for j in range(NBUF)]
        tsem = ec(nc.semaphore("tsem"))
        csem = ec(nc.semaphore("csem"))
        gsem = ec(nc.semaphore("gsem"))
        dmaP = ec(nc.semaphore("dmaP"))
        dmaG = ec(nc.semaphore("dmaG"))
        vsem = ec(nc.semaphore("vsem"))
        asem = ec(nc.semaphore("asem"))
        block = ec(nc.Block())

        # --- load list: 31 full tiles + the last tile split in two
        # halves (shortens the end-of-stream serial tail). Per load:
        # (block, col0, width, dve_cols).
        loads = []
        for b in range(RB):
            for c in range(NCH):
                if b == RB - 1 and c == NCH - 1:
                    h = COLCH // 2
                    d = D_DVE // 2
                    loads.append((b, c * COLCH, h, d))
                    loads.append((b, c * COLCH + h, h, d))
                else:
                    loads.append((b, c * COLCH, COLCH, D_DVE))
        NL = len(loads)
        blk_last = {}
        blk_cols = {}
        for i, (b, _, _, _) in enumerate(loads):
            blk_last[b] = i
            blk_cols.setdefault(b, []).append(i)

        # --- DVE op schedule (simulated) so producers know the vsem value
        # at which each load's reduce has completed.
        v_done = [0] * NL
        v = 0
        for i in range(NL):
            v += 1                    # reduce of load i
            v_done[i] = v
            if i % 2 == 1 and i // 2 < NHALF:
                v += 1                # count half-op
            if i == blk_last[loads[i][0]]:
                v += 2                # block combine: add + reduce
        V_STREAM = v
        a_done = [i + 1 for i in range(NL)]
        A_STREAM = NL
        # final DVE ops: w combine add, tv*w mul
        V_FINAL = V_STREAM + 2
        A_FINAL = A_STREAM + 2

        # per-buffer use ordinals for dmaL thresholds
        use_of = [i // NBUF + 1 for i in range(NL)]

        @block.sync
        def _(sync):
            for i in range(NL):
                b, c0, w, _ = loads[i]
                buf = i % NBUF
                if i >= NBUF:
                    sync.wait_ge(vsem, v_done[i - NBUF])
                    sync.wait_ge(asem, a_done[i - NBUF])
                sync.dma_start(
                    data[buf][:, 0:w],
                    x_ext[b * P : (b + 1) * P, c0 : c0 + w],
                ).then_inc(dmaL[buf], 16)
                if i == 0:
                    # tiny label preloads ride the HWDGE queue behind the
                    # first stream tile; yfb replication happens on PE
                    sync.dma_start(yf_sb[:, :], yf_ext[:, :]).then_inc(dmaP, 16)
                    sync.dma_start(ylf_sb[:, :], ylf_ext[:, :]).then_inc(dmaP, 16)
                    sync.dma_start(off_sb[:, :], off_ext[:, :]).then_inc(dmaP, 16)

        @block.gpsimd
        def _(gpsimd):
            gpsimd.memset(ones_sb[:, :], 1.0).then_inc(gsem, 1)
            # gathers issue late so SWDGE descriptor traffic stays off the
            # stream window; they still complete well before the final mult
            gpsimd.wait_ge(dmaP, 48)
            gpsimd.wait_ge(vsem, v_done[NL - 8])
            x_flat = x_ext[:, :].rearrange("a b -> (a b)").unsqueeze(1)
            for b in range(RB):
                gpsimd.indirect_dma_start(
                    out=tv[:, b : b + 1],
                    out_offset=None,
                    in_=x_flat,
                    in_offset=bass.IndirectOffsetOnAxis(
                        ap=off_sb[:, b : b + 1], axis=0
                    ),
                ).then_inc(dmaG, 16)
            gpsimd.wait_ge(asem, A_FINAL)
            gpsimd.dma_start(out_ext[:, :], acc[:, :]).then_inc(dmaG, 16)

        @block.tensor
        def _(tensor):
            tensor.wait_ge(dmaP, 48)
            tensor.wait_ge(gsem, 1)
            for j in range(N // 512):
                if j >= 2:
                    tensor.wait_ge(csem, j - 1)
                nc.tensor.matmul(
                    psum[j % 2][:, :],
                    lhsT=ones_sb[0:1, :],
                    rhs=yf_sb[0:1, j * 512 : (j + 1) * 512],
                    start=True,
                    stop=True,
                ).then_inc(tsem, 1)

        @block.vector
        def _(vector):
            # vv mirrors the vsem value as ops are emitted; a same-engine
            # RAW consumer must first wait_ge(vsem, vv) to flush in-flight
            # writes (DVE does not order back-to-back SBUF RAW by itself).
            vv = 0
            first_count = True
            for i in range(NL):
                b, c0, w, d = loads[i]
                buf = i % NBUF
                vector.wait_ge(dmaL[buf], 16 * use_of[i])
                nc.vector.reduce_sum(
                    rs_part[:, i : i + 1],
                    data[buf][:, 0:d],
                    axis=mybir.AxisListType.X,
                ).then_inc(vsem, 1)
                vv += 1
                if i % 2 == 1 and i // 2 < NHALF:
                    h = i // 2            # count half-op index
                    cb, hh = divmod(h, 2)
                    if first_count:
                        vector.wait_ge(dmaP, 48)
                        vector.wait_ge(csem, N // 512)
                        first_count = False
                    nc.vector.tensor_scalar(
                        out=eqscr[:, :],
                        in0=yfb[:, hh * HALF : (hh + 1) * HALF],
                        scalar1=ylf_sb[:, cb : cb + 1],
                        scalar2=None,
                        op0=mybir.AluOpType.is_equal,
                        op1=mybir.AluOpType.add,
                        accum_out=w_half[:, h : h + 1],
                    ).then_inc(vsem, 1)
                    vv += 1
                if i == blk_last[b]:
                    cols = blk_cols[b]
                    lo, hi = cols[0], cols[-1] + 1
                    vector.wait_ge(asem, i + 1)
                    vector.wait_ge(vsem, vv)  # flush rs_part writes
                    nc.vector.tensor_tensor(
                        out=sum4[:, 0 : hi - lo],
                        in0=rs_part[:, lo:hi],
                        in1=act_part[:, lo:hi],
                        op=mybir.AluOpType.add,
                    ).then_inc(vsem, 1)
                    vv += 1
                    vector.wait_ge(vsem, vv)  # flush sum4 write
                    nc.vector.reduce_sum(
                        rs[:, b : b + 1],
                        sum4[:, 0 : hi - lo],
                        axis=mybir.AxisListType.X,
                    ).then_inc(vsem, 1)
                    vv += 1
            # epilogue: combine count halves, then tprod = tv * w
            vector.wait_ge(vsem, vv)  # flush w_half writes
            nc.vector.tensor_tensor(
                out=w_sb[:, :],
                in0=w_half[:].rearrange("p (b t) -> p b t", t=2)[:, :, 0],
                in1=w_half[:].rearrange("p (b t) -> p b t", t=2)[:, :, 1],
                op=mybir.AluOpType.add,
            ).then_inc(vsem, 1)
            vv += 1
            vector.wait_ge(dmaG, 16 * RB)
            vector.wait_ge(vsem, vv)  # flush w_sb write
            nc.vector.tensor_tensor(
                out=tprod[:, :], in0=tv[:, :], in1=w_sb[:, :],
                op=mybir.AluOpType.mult,
            ).then_inc(vsem, 1)
            vv += 1
            assert vv == V_FINAL, (vv, V_FINAL)

        @block.scalar
        def _(scalar):
            for j in range(N // 512):
                scalar.wait_ge(tsem, j + 1)
                nc.scalar.activation(
                    out=yfb[:, j * 512 : (j + 1) * 512],
                    in_=psum[j % 2][:, :],
                    func=mybir.ActivationFunctionType.Copy,
                ).then_inc(csem, 1)
            for i in range(NL):
                _, _, w, d = loads[i]
                buf = i % NBUF
                scalar.wait_ge(dmaL[buf], 16 * use_of[i])
                nc.scalar.activation(
                    out=act_scr[:, 0 : w - d],
                    in_=data[buf][:, d:w],
                    func=mybir.ActivationFunctionType.Copy,
                    accum_out=act_part[:, i : i + 1],
                ).then_inc(asem, 1)
            scalar.wait_ge(vsem, V_FINAL)
            nc.scalar.activation(
                out=logt[:, :],
                in_=tprod[:, :],
                func=mybir.ActivationFunctionType.Ln,
                accum_out=acc[:, 0:1],
            ).then_inc(asem, 1)
            nc.scalar.activation(
                out=logt[:, :],
                in_=rs[:, :],
                func=mybir.ActivationFunctionType.Ln,
                accum_out=acc[:, 1:2],
            ).then_inc(asem, 1)

    return nc


def _get_nc():
    if "nc" not in _CACHE:
        _CACHE["nc"] = _build_nc()
    return _CACHE["nc"]


def _make_in_maps(output, y):
    out_f32 = np.ascontiguousarray(output, dtype=np.float32)
    y64 = np.asarray(y).astype(np.int64)
    yf = y64.astype(np.float32).reshape(1, N)
    in_maps = []
    for k in range(NCORES):
        rows = slice(k * RPC, (k + 1) * RPC)
        y_loc = y64[rows]
        # (p, b) layout: element (p, b) corresponds to local row b*128 + p
        ylf = np.ascontiguousarray(y_loc.astype(np.float32).reshape(RB, P).T)
        off = np.ascontiguousarray(
            (np.arange(RPC, dtype=np.int64) * C + y_loc)
            .astype(np.int32)
            .reshape(RB, P)
            .T
        )
        in_maps.append({"x": out_f32[rows], "yf": yf, "ylf": ylf, "off": off})
    return in_maps


def kernel(output, y):
    from concourse.bass_utils import run_bass_kernel_spmd

    output = np.asarray(output)
    y = np.asarray(y)
    assert output.shape == (N, C) and y.shape == (N,)

    in_maps = _make_in_maps(output, y)
    res = run_bass_kernel_spmd(
        _get_nc(), in_maps, core_ids=list(range(NCORES))
    )
    total = 0.0
    for k in range(NCORES):
        o = res.results[k]["out"]
        total += float(o[:, 0].sum(dtype=np.float64)) - float(
            o[:, 1].sum(dtype=np.float64)
        )
    loss = math.log(N) - total / N
    return np.float32(loss)
